# revision 1
# baseline (speedup 1.0000x reference)
"""DiffusionNetBlock on 8 Trainium2 NeuronCores.

Strategy (data-parallel over batch x row-halves, 8 cores = 4 batches x 2):
  core c = 2*b + h owns batch b and half of its mesh vertices.

Host-side prep (sharding/layout only, no model math beyond input folding):
  - fold vertex_areas into x_in, precompute the spectral heat scale
    exp(-evals x times) (tiny [K,P] per batch), transpose weights.
  - the sparse gradient (COO, E=160k edges/batch) is laid out for the
    device: rows of each batch are degree-sorted into 128-row blocks,
    blocks dealt to the two cores, and each block padded to a fixed
    per-slot degree D (equalized across cores so one NEFF serves all 8).
    Edges become dense fp8 streams xev = evecs[col] tiled as DoubleRow
    pairs [128 edges, 2 k-tiles, K]; the weighted segment-sum over rows
    is a 256-deep fp8 DoubleRow matmul with a selector carrying the
    gradX/gradY values, fully on the PE with f32 PSUM accumulation.

Device kernel (Bass/Tile, same program on all 8 cores):
  A: x_spec = evecs^T @ (a*x_in)          (PSUM accum over 157 chunks)
     s2 = exp(-lam t) * x_spec            (one DVE op)
  B: x_diffuse^T = s2^T @ evecs^T         (kept in SBUF, [P, rows])
  C: agX^T/agY^T per 128-row block via fp8 DoubleRow selector matmuls
     gx^T = s2^T @ agX^T, gy^T = s2^T @ agY^T
  D: xg = tanh(gx*(B_re gx) + gy*(B_im gy))
  E: 3-layer MLP on [x_in; x_diffuse; xg], + residual
  All of C-E runs in transposed [feature, row] layout in 512-row groups.
Host inverse-permutes/transposes the output.
"""

import math
import os
import sys

import ml_dtypes
import numpy as np

sys.path.insert(0, "/opt/trn_rl_repo")

from concourse import bass, mybir  # noqa: E402
from concourse import bass_utils  # noqa: E402
from concourse.tile import TileContext  # noqa: E402
from concourse.vector_clock import ScopedClock, VectorClock  # noqa: E402

B, N, P, K, E = 4, 20000, 128, 128, 160000
NCORES = 8
NBLK = 79                    # 128-row blocks per core
ROWS = NBLK * 128            # 10112 row slots per core
TOTBLK = 2 * NBLK            # 158 blocks per batch (20224 >= 20000 row slots)
GRP = 4                      # blocks per 512-wide processing group
NCHUNK = (N + 127) // 128    # 157 n-chunks for phase A (20096 padded)
NPAD = NCHUNK * 128

f32 = mybir.dt.float32
f32r = mybir.dt.float32r
f16 = mybir.dt.float16
f8 = mybir.dt.float8e4
f8np = ml_dtypes.float8_e4m3


# --------------------------------------------------------------- BIR fixup
# This toolchain's walrus encodes at most ONE sync wait per instruction
# ("Too many sync wait commands"), but Tile's add_semaphores freely
# attaches several. Hoist excess waits onto EventSemaphore carriers on
# the same engine, inserted just before the over-subscribed instruction.

def _split_excess_waits(bir_json: bytes) -> bytes:
    import json
    d = json.loads(bir_json)
    n_split = 0
    for fn in d.get("functions", []):
        for blk in fn.get("blocks", []):
            insts = blk.get("instructions")
            if not insts:
                continue
            out = []
            changed = False
            for ins in insts:
                si = ins.get("sync_info") or {}
                ow = si.get("on_wait") or []
                if len(ow) > 1 and "engine" in ins:
                    for w in ow[:-1]:
                        n_split += 1
                        out.append({
                            "debug": ins.get("debug", 0),
                            "engine": ins["engine"],
                            "ins": [],
                            "outs": [],
                            "name": f"{ins['name']}-xw{n_split}",
                            "opcode": "EventSemaphore",
                            "sync_info": {"on_update": [], "on_wait": [w]},
                        })
                    si["on_wait"] = [ow[-1]]
                    changed = True
                out.append(ins)
            if changed:
                blk["instructions"] = out
    if n_split == 0:
        return bir_json
    return json.dumps(d).encode()


_orig_compile_bir_kernel = bass_utils.compile_bir_kernel


def _patched_compile_bir_kernel(bir_json, tmpdir, neff_name="file.neff"):
    return _orig_compile_bir_kernel(_split_excess_waits(bir_json), tmpdir,
                                    neff_name)


def _install_birfix():
    from concourse import bass2jax
    if bass_utils.compile_bir_kernel.__name__ != "_patched_compile_bir_kernel":
        bass_utils.compile_bir_kernel = _patched_compile_bir_kernel
    if bass2jax.compile_bir_kernel.__name__ != "_patched_compile_bir_kernel":
        bass2jax.compile_bir_kernel = _patched_compile_bir_kernel


_install_birfix()


class FixedTileContext(TileContext):
    """Stock _drain_and_barrier stuffs every outstanding sem wait onto one
    SP Drain; TRN2 TPB_CTRL encoding only fits 1-2 sync waits and walrus
    dies with "Too many sync wait commands". Split the final global-clock
    wait into one Drain per logical proc."""

    def _drain_and_barrier(self, tick_clock, wait_clock):
        gc = tick_clock.global_clock
        n = len(gc)
        for p in range(n):
            if gc[p] > 0:
                vec = [0] * n
                vec[p] = gc[p]
                w = self.nc.sync.drain()
                wait_clock.add_sem_waits(w.ins, ScopedClock({None: VectorClock(vec)}))
        # The per-proc drains above run serially on SP, so every wait is
        # already satisfied here; emit the final drain bare.
        self.nc.sync.drain()
        self.nc.all_engine_barrier()
        assert self.sems is not None
        popped = self.nc._tile_sem_poison_stack.pop()
        assert popped is self._sem_poison
        self.nc.clear_and_free_semaphores(list(self.sems.allocated().values()))
        self.nc.all_engine_barrier()


# ---------------------------------------------------------------- host prep


def _plan_slots(grad_rows):
    """Degree-sort rows per batch into blocks, deal to cores, and compute
    the global per-slot degree D (equalized across all 8 cores)."""
    perms = []          # per batch: [TOTBLK*128] row ids (-1 = pad)
    degs = []
    d_blocks = np.zeros((B, 2, NBLK), np.int64)
    for b in range(B):
        deg = np.bincount(np.asarray(grad_rows[b]), minlength=N)
        order = np.argsort(-deg, kind="stable")
        perm = np.concatenate([order, np.full(TOTBLK * 128 - N, -1, np.int64)])
        dblk = deg[np.maximum(perm, 0)] * (perm >= 0)
        dblk = dblk.reshape(TOTBLK, 128).max(axis=1)
        for i in range(TOTBLK):
            d_blocks[b, i % 2, i // 2] = dblk[i]
        perms.append(perm)
        degs.append(deg)
    d_slots = np.maximum(d_blocks.max(axis=(0, 1)), 1)   # [NBLK]
    assert d_slots.max() <= 128, d_slots.max()
    return perms, degs, d_slots


def _slot_geometry(d_slots):
    """fp8 DoubleRow geometry. Per slot: half-degree dh (edges per row per
    k-tile), rows-per-instruction R, instructions T, stream tile offset,
    and the column offset of this slot's selector block (4*R*T cols: per
    tile, per k-tile half, [selX_R | selY_R])."""
    geo = []
    t_off = 0
    s_off = 0
    for D in d_slots.tolist():
        dh = (D + 1) // 2
        R = 128 // dh
        T = math.ceil(128 / R)
        geo.append((dh, R, T, t_off, s_off))
        t_off += T
        s_off += 4 * T * R
    return geo, t_off, s_off


def build_host_data(inputs):
    x_in = np.asarray(inputs["x_in"], np.float32)
    areas = np.asarray(inputs["vertex_areas"], np.float32)
    evals = np.asarray(inputs["evals"], np.float32)
    evecs = np.asarray(inputs["evecs"], np.float32)
    gxv = np.asarray(inputs["gradX_vals"], np.float32)
    gyv = np.asarray(inputs["gradY_vals"], np.float32)
    grows = np.asarray(inputs["grad_rows"], np.int64)
    gcols = np.asarray(inputs["grad_cols"], np.int64)
    times = np.clip(np.asarray(inputs["diffusion_times"], np.float32), 1e-8, None)
    W1 = np.asarray(inputs["W1"], np.float32)
    b1 = np.asarray(inputs["b1"], np.float32)
    W2 = np.asarray(inputs["W2"], np.float32)
    b2 = np.asarray(inputs["b2"], np.float32)
    W3 = np.asarray(inputs["W3"], np.float32)
    b3 = np.asarray(inputs["b3"], np.float32)
    B_re = np.asarray(inputs["B_re"], np.float32)
    B_im = np.asarray(inputs["B_im"], np.float32)

    perms, degs, d_slots = _plan_slots(grows)
    geo, TT, SELTOT = _slot_geometry(d_slots)

    # phase A inputs, partition-major: ax[p, c, 0, :] = evecs row c*128+p,
    # ax[p, c, 1, :] = (a*x_in) row c*128+p
    ax_all = np.zeros((B, NPAD, 2, P), np.float16)
    ax_all[:, :N, 0, :] = evecs.astype(np.float16)
    ax_all[:, :N, 1, :] = (x_in * areas[:, :, None]).astype(np.float16)
    ax_all = np.ascontiguousarray(
        ax_all.reshape(B, NCHUNK, 128, 2, P).transpose(0, 2, 1, 3, 4))

    in_maps = []
    core_perm = []
    for b in range(B):
        rows_b, cols_b = grows[b], gcols[b]
        esort = np.argsort(rows_b, kind="stable")
        deg = degs[b]
        rowptr = np.zeros(N + 1, np.int64)
        rowptr[1:] = np.cumsum(deg)
        scale = np.exp(-evals[b][:, None] * times[None, :]).astype(np.float32)
        ev8 = evecs[b].astype(f8np)
        for h in range(2):
            blk_ids = 2 * np.arange(NBLK) + h          # block index within batch
            perm_own = perms[b].reshape(TOTBLK, 128)[blk_ids].reshape(-1)  # [ROWS]
            core_perm.append(perm_own)
            pv = np.maximum(perm_own, 0)
            valid = perm_own >= 0

            # per-row padded edge grid, slot by slot, split into 2 k-tile
            # halves for fp8 DoubleRow (256-edge contraction / instruction)
            col_stream = np.zeros((TT, 2, 128), np.int64)
            selxy = np.zeros((128, SELTOT), f8np)
            for s, (dh, R, T, toff, soff) in enumerate(geo):
                rows_blk = perm_own[s * 128:(s + 1) * 128]
                rb = np.maximum(rows_blk, 0)
                cnt = np.where(rows_blk >= 0, deg[rb], 0)
                c0 = (cnt + 1) // 2                      # ceil half
                starts = [np.zeros_like(c0), c0]
                halves = [c0, cnt - c0]
                G = T * R
                e = np.arange(128)
                ei, ed = e // dh, e % dh             # row-in-tile, slot-in-row
                emask = ei < R
                eis = np.where(emask, ei, 0)
                sel_t = np.zeros((T, 2, 128, 2 * R), np.float32)
                for hh in range(2):
                    idx = (rowptr[rb][:, None] + starts[hh][:, None]
                           + np.arange(dh)[None, :])
                    mask = np.arange(dh)[None, :] < halves[hh][:, None]
                    eid = esort[np.where(mask, idx, 0)]
                    cm = np.where(mask, cols_b[eid], 0)          # [128, dh]
                    vxm = np.where(mask, gxv[b][eid], 0.0)
                    vym = np.where(mask, gyv[b][eid], 0.0)
                    pad = ((0, G - 128), (0, 0))
                    cmp_ = np.pad(cm, pad).reshape(T, R * dh)
                    col_stream[toff:toff + T, hh] = np.pad(
                        cmp_, ((0, 0), (0, 128 - R * dh)))
                    vxm = np.pad(vxm, pad).reshape(T, R, dh)
                    vym = np.pad(vym, pad).reshape(T, R, dh)
                    sel_t[:, hh, e, eis] = vxm[:, eis, ed] * emask
                    sel_t[:, hh, e, R + eis] = vym[:, eis, ed] * emask
                selxy[:, soff:soff + 4 * T * R] = (
                    sel_t.transpose(2, 0, 1, 3).reshape(128, T * 4 * R)
                    .astype(f8np))

            evg = np.ascontiguousarray(
                ev8[col_stream].transpose(2, 0, 1, 3))   # [128, TT, 2, K]

            in_maps.append({
                "evg": evg,
                "selxy": selxy,
                "ax": ax_all[b],
                "evsT": np.ascontiguousarray(
                    (evecs[b][pv].T * valid[None, :]).astype(np.float16)),
                "xinT": np.ascontiguousarray(
                    (x_in[b][pv].T * valid[None, :]).astype(np.float16)),
                "scale": scale,
                "w1t": np.ascontiguousarray(W1.T.reshape(3, P, P).astype(np.float16)),
                "w2t": np.ascontiguousarray(W2.T.astype(np.float16)),
                "w3t": np.ascontiguousarray(W3.T.astype(np.float16)),
                "bret": np.ascontiguousarray(B_re.T.astype(np.float16)),
                "bimt": np.ascontiguousarray(B_im.T.astype(np.float16)),
                "b1": b1.reshape(P, 1).copy(),
                "b2": b2.reshape(P, 1).copy(),
                "b3": b3.reshape(P, 1).copy(),
            })

    meta = {"geo": geo, "TT": TT, "SELTOT": SELTOT, "d_slots": d_slots}
    return in_maps, core_perm, meta


# ------------------------------------------------------------ device kernel


def build_bass(meta):
    geo = meta["geo"]
    TT = meta["TT"]
    SELTOT = meta["SELTOT"]

    nc = bass.Bass("TRN2", target_bir_lowering=False, debug=False,
                   num_devices=NCORES)

    evg_d = nc.dram_tensor("evg", [128, TT, 2, K], f8, kind="ExternalInput")
    selxy_d = nc.dram_tensor("selxy", [128, SELTOT], f8, kind="ExternalInput")
    ax_d = nc.dram_tensor("ax", [128, NCHUNK, 2, P], f16, kind="ExternalInput")
    evsT_d = nc.dram_tensor("evsT", [K, ROWS], f16, kind="ExternalInput")
    xinT_d = nc.dram_tensor("xinT", [P, ROWS], f16, kind="ExternalInput")
    scale_d = nc.dram_tensor("scale", [K, P], f32, kind="ExternalInput")
    w1t_d = nc.dram_tensor("w1t", [3, P, P], f16, kind="ExternalInput")
    w2t_d = nc.dram_tensor("w2t", [P, P], f16, kind="ExternalInput")
    w3t_d = nc.dram_tensor("w3t", [P, P], f16, kind="ExternalInput")
    bret_d = nc.dram_tensor("bret", [P, P], f16, kind="ExternalInput")
    bimt_d = nc.dram_tensor("bimt", [P, P], f16, kind="ExternalInput")
    b1_d = nc.dram_tensor("b1", [P, 1], f32, kind="ExternalInput")
    b2_d = nc.dram_tensor("b2", [P, 1], f32, kind="ExternalInput")
    b3_d = nc.dram_tensor("b3", [P, 1], f32, kind="ExternalInput")
    outT_d = nc.dram_tensor("outT", [P, ROWS], f16, kind="ExternalOutput")

    AF = mybir.ActivationFunctionType
    DR = mybir.MatmulPerfMode.DoubleRow
    XCH = 16       # evg DoubleRow tiles per DMA chunk
    EVG_BUFS = 8   # evg ring depth (chunks)

    with FixedTileContext(nc) as tc:
        with (
            tc.tile_pool(name="consts", bufs=1) as cpool,
            tc.tile_pool(name="xdpool", bufs=1) as xdpool,
            tc.tile_pool(name="pX", bufs=EVG_BUFS) as pX,
        ):
            scale_t = cpool.tile([K, P], f32, tag="scale")
            nc.scalar.dma_start(scale_t[:], scale_d[:])
            wh = cpool.tile([P, 7, P], f16, tag="wh")
            nc.scalar.dma_start(wh[:, 0:3, :], w1t_d[:].rearrange("s p q -> p s q"))
            nc.scalar.dma_start(wh[:, 3, :], w2t_d[:])
            nc.scalar.dma_start(wh[:, 4, :], w3t_d[:])
            nc.scalar.dma_start(wh[:, 5, :], bret_d[:])
            nc.scalar.dma_start(wh[:, 6, :], bimt_d[:])
            w1t_t = wh[:, 0:3, :]
            w2t_t = wh[:, 3, :]
            w3t_t = wh[:, 4, :]
            bret_t = wh[:, 5, :]
            bimt_t = wh[:, 6, :]
            b1_t = cpool.tile([P, 1], f32, tag="b1")
            nc.scalar.dma_start(b1_t[:], b1_d[:])
            b2_t = cpool.tile([P, 1], f32, tag="b2")
            nc.scalar.dma_start(b2_t[:], b2_d[:])
            b3_t = cpool.tile([P, 1], f32, tag="b3")
            nc.scalar.dma_start(b3_t[:], b3_d[:])
            s2h_t = cpool.tile([K, P], f16, tag="s2h")
            xdT_t = xdpool.tile([P, ROWS], f16, tag="xdT")
            xinT_t = xdpool.tile([P, ROWS], f16, tag="xinT")

            # eager evg prefetch: the whole stream is an input, so issue
            # every chunk DMA up front on gpsimd (which carries nothing
            # else — the ring's WAR waits stall only this queue) and let
            # the pool's ring recycling pace it against consumption.
            evg_chunks = []
            for t0 in range(0, TT, XCH):
                w = min(XCH, TT - t0)
                xt = pX.tile([128, XCH, 2, K], f8, tag="evg")
                nc.gpsimd.dma_start(xt[:, :w], evg_d[:, t0:t0 + w])
                evg_chunks.append(xt)

            def evg_tile(t):
                return evg_chunks[t // XCH][:, t % XCH, :, :]

            # ---------------- phase A: x_spec, s2
            ACH = 8
            with (
                tc.tile_pool(name="pA", bufs=6) as pA,
                tc.tile_pool(name="psA", bufs=1, space="PSUM") as psA_pool,
            ):
                psA = psA_pool.tile([K, P], f32, tag="psA")
                for c0 in range(0, NCHUNK, ACH):
                    w = min(ACH, NCHUNK - c0)
                    ax_t = pA.tile([128, ACH, 2, P], f16, tag="axA")
                    nc.sync.dma_start(ax_t[:, :w], ax_d[:, c0:c0 + w])
                    for i in range(w):
                        nc.tensor.matmul(
                            psA[:], ax_t[:, i, 0, :], ax_t[:, i, 1, :],
                            start=(c0 + i == 0), stop=(c0 + i == NCHUNK - 1),
                        )
                nc.vector.tensor_mul(s2h_t[:], scale_t[:], psA[:])

            # ---------------- phase B: x_diffuse^T resident in SBUF
            with (
                tc.tile_pool(name="pB", bufs=10) as pB,
                tc.tile_pool(name="psB", bufs=2, space="PSUM") as psB_pool,
            ):
                for g0 in range(0, ROWS, 512):
                    w = min(512, ROWS - g0)
                    evsT_t = pB.tile([K, 512], f16, tag="evsTB")
                    nc.sync.dma_start(evsT_t[:, :w], evsT_d[:, g0:g0 + w])
                    psB = psB_pool.tile([P, 512], f32, tag="psB")
                    nc.tensor.matmul(
                        psB[:, :w], s2h_t[:],
                        evsT_t[:, :w], start=True, stop=True,
                    )
                    nc.scalar.activation(xdT_t[:, g0:g0 + w], psB[:, :w], AF.Copy)

            # xinT arrives during early C (first use: E of group 0)
            nc.sync.dma_start(xinT_t[:], xinT_d[:])

            # ---------------- phases C-E per 512-row group
            SCR = 64       # de-interleave overrun scratch columns
            SEL_LA = 4     # selg prefetch lookahead (groups)
            mxagg = max(T * R for (dh, R, T, _, _) in geo)
            mxsel = max(4 * T * R for (dh, R, T, _, _) in geo)
            with (
                tc.tile_pool(name="pS", bufs=SEL_LA + 2) as pS,
                tc.tile_pool(name="pG", bufs=2) as pG,
                tc.tile_pool(name="psAG", bufs=2, space="PSUM") as psAG_pool,
                tc.tile_pool(name="psGXY", bufs=1, space="PSUM") as psGXY_pool,
                tc.tile_pool(name="psBXY", bufs=1, space="PSUM") as psBXY_pool,
                tc.tile_pool(name="psH", bufs=2, space="PSUM") as psH_pool,
            ):
                groups = list(range(0, NBLK, GRP))
                sel_tiles = {}

                def sel_fetch(g):
                    nb = min(GRP, NBLK - g)
                    sel0 = geo[g][4]
                    sel1 = (geo[g + nb][4] if g + nb < NBLK else SELTOT)
                    selg = pS.tile([128, GRP * mxsel], f8, tag="selg")
                    nc.sync.dma_start(selg[:, :sel1 - sel0],
                                      selxy_d[:, sel0:sel1])
                    sel_tiles[g] = selg

                for g in groups[:SEL_LA]:
                    sel_fetch(g)

                for gi, g in enumerate(groups):
                    if gi + SEL_LA < len(groups):
                        sel_fetch(groups[gi + SEL_LA])
                    nb = min(GRP, NBLK - g)
                    gw = nb * 128
                    g0 = g * 128
                    sel0 = geo[g][4]
                    selg = sel_tiles.pop(g)
                    agXY_sb = pG.tile([K, 2, GRP * 128 + SCR], f16, tag="agxy")
                    for q in range(nb):
                        s = g + q
                        dh, R, T, toff, soff = geo[s]
                        so = soff - sel0
                        # strided matmul out: X cols land in [:, 0, jR:...],
                        # Y cols in [:, 1, jR:...] -> row-contiguous halves
                        # with no de-interleave pass.
                        agXY = psAG_pool.tile([K, 2, mxagg], f32, tag="agXY")
                        for j in range(T):
                            nc.tensor.matmul(
                                agXY[:, :, j * R:(j + 1) * R],
                                evg_tile(toff + j),
                                selg[:, so + 4 * j * R:so + 4 * (j + 1) * R]
                                .rearrange("k (h x) -> k h x", h=2),
                                start=True, stop=True, perf_mode=DR,
                            )
                        if q % 2 == 0:
                            nc.vector.tensor_copy(
                                agXY_sb[:, :, q * 128:q * 128 + T * R],
                                agXY[:, :, :T * R])
                        else:
                            nc.scalar.copy(
                                agXY_sb[:, :, q * 128:q * 128 + T * R],
                                agXY[:, :, :T * R])

                    # C2: gx^T, gy^T
                    psGXY = psGXY_pool.tile([P, 2, GRP * 128], f32, tag="psGXY")
                    nc.tensor.matmul(psGXY[:, 0, :gw], s2h_t[:],
                                     agXY_sb[:, 0, :gw], start=True, stop=True)
                    nc.tensor.matmul(psGXY[:, 1, :gw], s2h_t[:],
                                     agXY_sb[:, 1, :gw], start=True, stop=True)
                    gxy_sb = pG.tile([P, 2, GRP * 128], f16, tag="gxy")
                    nc.vector.tensor_copy(gxy_sb[:, 0, :gw], psGXY[:, 0, :gw])
                    nc.scalar.copy(gxy_sb[:, 1, :gw], psGXY[:, 1, :gw])

                    # D: xg = tanh(gx*(B_re gx) + gy*(B_im gy))
                    psBXY = psBXY_pool.tile([P, 2, GRP * 128], f32, tag="psBXY")
                    nc.tensor.matmul(psBXY[:, 0, :gw], bret_t[:],
                                     gxy_sb[:, 0, :gw], start=True, stop=True)
                    nc.tensor.matmul(psBXY[:, 1, :gw], bimt_t[:],
                                     gxy_sb[:, 1, :gw], start=True, stop=True)
                    t1 = pG.tile([P, 2, GRP * 128], f32, tag="t1")
                    nc.vector.tensor_mul(t1[:, :, :gw], gxy_sb[:, :, :gw],
                                         psBXY[:, :, :gw])
                    t2 = pG.tile([P, GRP * 128], f32, tag="t2")
                    nc.vector.tensor_add(t2[:, :gw], t1[:, 0, :gw],
                                         t1[:, 1, :gw])
                    xg_sb = pG.tile([P, GRP * 128], f16, tag="xg")
                    nc.scalar.activation(xg_sb[:, :gw], t2[:, :gw], AF.Tanh)

                    # E: MLP + residual
                    psH1 = psH_pool.tile([P, GRP * 128], f32, tag="psH")
                    nc.tensor.matmul(psH1[:, :gw], w1t_t[:, 0, :],
                                     xinT_t[:, g0:g0 + gw],
                                     start=True, stop=False)
                    nc.tensor.matmul(psH1[:, :gw], w1t_t[:, 1, :],
                                     xdT_t[:, g0:g0 + gw],
                                     start=False, stop=False)
                    nc.tensor.matmul(psH1[:, :gw], w1t_t[:, 2, :],
                                     xg_sb[:, :gw], start=False, stop=True)
                    h_sb = pG.tile([P, GRP * 128], f16, tag="h")
                    nc.scalar.activation(h_sb[:, :gw], psH1[:, :gw], AF.Relu,
                                         bias=b1_t[:])
                    psH2 = psH_pool.tile([P, GRP * 128], f32, tag="psH")
                    nc.tensor.matmul(psH2[:, :gw], w2t_t[:],
                                     h_sb[:, :gw], start=True, stop=True)
                    h2_sb = pG.tile([P, GRP * 128], f16, tag="h")
                    nc.scalar.activation(h2_sb[:, :gw], psH2[:, :gw], AF.Relu,
                                         bias=b2_t[:])
                    psH3 = psH_pool.tile([P, GRP * 128], f32, tag="psH")
                    nc.tensor.matmul(psH3[:, :gw], w3t_t[:],
                                     h2_sb[:, :gw], start=True, stop=True)
                    out_sb = pG.tile([P, GRP * 128], f16, tag="out")
                    nc.vector.scalar_tensor_tensor(
                        out_sb[:, :gw], psH3[:, :gw], b3_t[:],
                        xinT_t[:, g0:g0 + gw],
                        op0=mybir.AluOpType.add, op1=mybir.AluOpType.add)
                    nc.sync.dma_start(outT_d[:, g0:g0 + gw], out_sb[:, :gw])

    return nc


# ---------------------------------------------------------------- top level

_CACHE = {}


def _get_bass(meta):
    key = tuple(meta["d_slots"].tolist())
    if key not in _CACHE:
        _CACHE[key] = build_bass(meta)
    return _CACHE[key]


def kernel(_trace=False, **inputs):
    in_maps, core_perm, meta = build_host_data(inputs)
    nc = _get_bass(meta)
    res = bass_utils.run_bass_kernel_spmd(
        nc, in_maps, core_ids=list(range(NCORES)), trace=_trace,
        trace_cores=list(range(NCORES)) if _trace else None,
    )
    out = np.zeros((B, N, P), np.float32)
    for c in range(NCORES):
        b = c // 2
        perm = core_perm[c]
        valid = perm >= 0
        outT = res.results[c]["outT"]           # [P, ROWS]
        out[b, perm[valid]] = np.asarray(outT, np.float32).T[valid]
    if _trace:
        return out, res
    return out



# revision 23
# speedup vs baseline: 1.1511x; 1.1511x over previous
"""DiffusionNetBlock on 8 Trainium2 NeuronCores.

Strategy (data-parallel over batch x row-halves, 8 cores = 4 batches x 2):
  core c = 2*b + h owns batch b and half of its mesh vertices.

Host-side prep (sharding/layout only, no model math beyond input folding):
  - fold vertex_areas into x_in, precompute the spectral heat scale
    exp(-evals x times) (tiny [K,P] per batch), transpose weights.
  - the sparse gradient (COO, E=160k edges/batch) is laid out for the
    device: rows of each batch are degree-sorted into 128-row blocks,
    blocks dealt to the two cores, and each block padded to a fixed
    per-slot degree D (equalized across cores so one NEFF serves all 8).
    Edges become dense fp8 streams xev = evecs[col] tiled as DoubleRow
    pairs [128 edges, 2 k-tiles, K]; the weighted segment-sum over rows
    is a 256-deep fp8 DoubleRow matmul with a selector carrying the
    gradX/gradY values, fully on the PE with f32 PSUM accumulation.

Device kernel (Bass/Tile, same program on all 8 cores), v2 schedule:
  A: x_spec partial = evecs^T @ (a*x_in) over OWN half of rows only;
     64KB AllReduce across the core pair completes x_spec, then
     s2 = exp(-lam t) * x_spec, and Mre = s2 @ B_re^T, Mim = s2 @ B_im^T
     are derived on device (fusing the SpatialGradient mixing into C2).
  C1: agX^T/agY^T per 128-row block via fp8 DoubleRow selector matmuls,
     streamed from t=0 (independent of A), staged to SBUF for all groups.
  Per 512-row group (software-pipelined; E is spread over 3 iterations
  so PE never stalls on the scalar-engine relu/tanh latencies):
     B:  x_diffuse^T = s2^T @ evecs^T
     C2: gx^T = s2^T agX, gy^T = s2^T agY, bx^T = Mre^T agX,
         by^T = Mim^T agY   (all directly from the staged agXY)
     D:  xg = tanh(gx*bx + gy*by)
     E:  3-layer MLP on [x_in; x_diffuse; xg], + residual
  Host inverse-permutes/transposes the output.
"""

import math
import os
import sys

import ml_dtypes
import numpy as np

sys.path.insert(0, "/opt/trn_rl_repo")

from concourse import bass, mybir  # noqa: E402
from concourse import bass_utils  # noqa: E402
from concourse.tile import TileContext  # noqa: E402
from concourse.vector_clock import ScopedClock, VectorClock  # noqa: E402

B, N, P, K, E = 4, 20000, 128, 128, 160000
NCORES = 8
NBLK = 79                    # 128-row blocks per core
ROWS = NBLK * 128            # 10112 row slots per core
TOTBLK = 2 * NBLK            # 158 blocks per batch (20224 >= 20000 row slots)
GRP = 4                      # blocks per 512-wide processing group
NG = (NBLK + GRP - 1) // GRP  # 20 groups (last one 3 blocks)
NCHUNK = (N + 127) // 128    # 157 n-chunks for phase A (20096 padded)
APAD = 2 * ((NCHUNK + 1) // 2)   # 158, so each core takes 79
NPAD = APAD * 128

USE_COLLECTIVE = False
LAGC = 8                     # groups C2D trails C1 (covers phase-A latency)

f32 = mybir.dt.float32
f32r = mybir.dt.float32r
f16 = mybir.dt.float16
f8 = mybir.dt.float8e4
f8np = ml_dtypes.float8_e4m3


# --------------------------------------------------------------- BIR fixup
# This toolchain's walrus encodes at most ONE sync wait per instruction
# ("Too many sync wait commands"), but Tile's add_semaphores freely
# attaches several. Hoist excess waits onto EventSemaphore carriers on
# the same engine, inserted just before the over-subscribed instruction.

def _split_excess_waits(bir_json: bytes) -> bytes:
    import json
    d = json.loads(bir_json)
    n_split = 0
    for fn in d.get("functions", []):
        for blk in fn.get("blocks", []):
            insts = blk.get("instructions")
            if not insts:
                continue
            out = []
            changed = False
            for ins in insts:
                si = ins.get("sync_info") or {}
                ow = si.get("on_wait") or []
                if len(ow) > 1 and "engine" in ins:
                    for w in ow[:-1]:
                        n_split += 1
                        out.append({
                            "debug": ins.get("debug", 0),
                            "engine": ins["engine"],
                            "ins": [],
                            "outs": [],
                            "name": f"{ins['name']}-xw{n_split}",
                            "opcode": "EventSemaphore",
                            "sync_info": {"on_update": [], "on_wait": [w]},
                        })
                    si["on_wait"] = [ow[-1]]
                    changed = True
                out.append(ins)
            if changed:
                blk["instructions"] = out
    if n_split == 0:
        return bir_json
    return json.dumps(d).encode()


_orig_compile_bir_kernel = bass_utils.compile_bir_kernel


def _patched_compile_bir_kernel(bir_json, tmpdir, neff_name="file.neff"):
    return _orig_compile_bir_kernel(_split_excess_waits(bir_json), tmpdir,
                                    neff_name)


def _install_birfix():
    from concourse import bass2jax
    if bass_utils.compile_bir_kernel.__name__ != "_patched_compile_bir_kernel":
        bass_utils.compile_bir_kernel = _patched_compile_bir_kernel
    if bass2jax.compile_bir_kernel.__name__ != "_patched_compile_bir_kernel":
        bass2jax.compile_bir_kernel = _patched_compile_bir_kernel


_install_birfix()


class FixedTileContext(TileContext):
    """Stock _drain_and_barrier stuffs every outstanding sem wait onto one
    SP Drain; TRN2 TPB_CTRL encoding only fits 1-2 sync waits and walrus
    dies with "Too many sync wait commands". Split the final global-clock
    wait into one Drain per logical proc."""

    def _drain_and_barrier(self, tick_clock, wait_clock):
        gc = tick_clock.global_clock
        n = len(gc)
        for p in range(n):
            if gc[p] > 0:
                vec = [0] * n
                vec[p] = gc[p]
                w = self.nc.sync.drain()
                wait_clock.add_sem_waits(w.ins, ScopedClock({None: VectorClock(vec)}))
        # The per-proc drains above run serially on SP, so every wait is
        # already satisfied here; emit the final drain bare.
        self.nc.sync.drain()
        self.nc.all_engine_barrier()
        assert self.sems is not None
        popped = self.nc._tile_sem_poison_stack.pop()
        assert popped is self._sem_poison
        self.nc.clear_and_free_semaphores(list(self.sems.allocated().values()))
        self.nc.all_engine_barrier()


# ---------------------------------------------------------------- host prep


def _plan_slots(grad_rows):
    """Degree-sort rows per batch into blocks, deal to cores, and compute
    the global per-slot degree D (equalized across all 8 cores)."""
    perms = []          # per batch: [TOTBLK*128] row ids (-1 = pad)
    degs = []
    d_blocks = np.zeros((B, 2, NBLK), np.int64)
    for b in range(B):
        deg = np.bincount(np.asarray(grad_rows[b]), minlength=N)
        order = np.argsort(-deg, kind="stable")
        perm = np.concatenate([order, np.full(TOTBLK * 128 - N, -1, np.int64)])
        dblk = deg[np.maximum(perm, 0)] * (perm >= 0)
        dblk = dblk.reshape(TOTBLK, 128).max(axis=1)
        for i in range(TOTBLK):
            d_blocks[b, i % 2, i // 2] = dblk[i]
        perms.append(perm)
        degs.append(deg)
    d_slots = np.maximum(d_blocks.max(axis=(0, 1)), 1)   # [NBLK]
    assert d_slots.max() <= 128, d_slots.max()
    return perms, degs, d_slots


def _slot_geometry(d_slots):
    """fp8 DoubleRow geometry. Per slot: half-degree dh (edges per row per
    k-tile), rows-per-instruction R, instructions T, stream tile offset,
    and the column offset of this slot's selector block (4*R*T cols: per
    tile, per k-tile half, [selX_R | selY_R])."""
    geo = []
    t_off = 0
    s_off = 0
    for D in d_slots.tolist():
        dh = (D + 1) // 2
        R = 128 // dh
        T = math.ceil(128 / R)
        geo.append((dh, R, T, t_off, s_off))
        t_off += T
        s_off += 4 * T * R
    return geo, t_off, s_off


def build_host_data(inputs):
    x_in = np.asarray(inputs["x_in"], np.float32)
    areas = np.asarray(inputs["vertex_areas"], np.float32)
    evals = np.asarray(inputs["evals"], np.float32)
    evecs = np.asarray(inputs["evecs"], np.float32)
    gxv = np.asarray(inputs["gradX_vals"], np.float32)
    gyv = np.asarray(inputs["gradY_vals"], np.float32)
    grows = np.asarray(inputs["grad_rows"], np.int64)
    gcols = np.asarray(inputs["grad_cols"], np.int64)
    times = np.clip(np.asarray(inputs["diffusion_times"], np.float32), 1e-8, None)
    W1 = np.asarray(inputs["W1"], np.float32)
    b1 = np.asarray(inputs["b1"], np.float32)
    W2 = np.asarray(inputs["W2"], np.float32)
    b2 = np.asarray(inputs["b2"], np.float32)
    W3 = np.asarray(inputs["W3"], np.float32)
    b3 = np.asarray(inputs["b3"], np.float32)
    B_re = np.asarray(inputs["B_re"], np.float32)
    B_im = np.asarray(inputs["B_im"], np.float32)

    perms, degs, d_slots = _plan_slots(grows)
    geo, TT, SELTOT = _slot_geometry(d_slots)

    # phase A inputs, partition-major: ax[p, c, 0, :] = evecs row c*128+p,
    # ax[p, c, 1, :] = (a*x_in) row c*128+p
    ax_all = np.zeros((B, NPAD, 2, P), np.float16)
    ax_all[:, :N, 0, :] = evecs.astype(np.float16)
    ax_all[:, :N, 1, :] = (x_in * areas[:, :, None]).astype(np.float16)
    ax_all = np.ascontiguousarray(
        ax_all.reshape(B, APAD, 128, 2, P).transpose(0, 2, 1, 3, 4))

    in_maps = []
    core_perm = []
    for b in range(B):
        rows_b, cols_b = grows[b], gcols[b]
        esort = np.argsort(rows_b, kind="stable")
        deg = degs[b]
        rowptr = np.zeros(N + 1, np.int64)
        rowptr[1:] = np.cumsum(deg)
        scale = np.exp(-evals[b][:, None] * times[None, :]).astype(np.float32)
        ev8 = evecs[b].astype(f8np)
        for h in range(2):
            blk_ids = 2 * np.arange(NBLK) + h          # block index within batch
            perm_own = perms[b].reshape(TOTBLK, 128)[blk_ids].reshape(-1)  # [ROWS]
            core_perm.append(perm_own)
            pv = np.maximum(perm_own, 0)
            valid = perm_own >= 0

            # per-row padded edge grid, slot by slot, split into 2 k-tile
            # halves for fp8 DoubleRow (256-edge contraction / instruction)
            col_stream = np.zeros((TT, 2, 128), np.int64)
            selxy = np.zeros((128, SELTOT), f8np)
            for s, (dh, R, T, toff, soff) in enumerate(geo):
                rows_blk = perm_own[s * 128:(s + 1) * 128]
                rb = np.maximum(rows_blk, 0)
                cnt = np.where(rows_blk >= 0, deg[rb], 0)
                c0 = (cnt + 1) // 2                      # ceil half
                starts = [np.zeros_like(c0), c0]
                halves = [c0, cnt - c0]
                G = T * R
                e = np.arange(128)
                ei, ed = e // dh, e % dh             # row-in-tile, slot-in-row
                emask = ei < R
                eis = np.where(emask, ei, 0)
                sel_t = np.zeros((T, 2, 128, 2 * R), np.float32)
                for hh in range(2):
                    idx = (rowptr[rb][:, None] + starts[hh][:, None]
                           + np.arange(dh)[None, :])
                    mask = np.arange(dh)[None, :] < halves[hh][:, None]
                    eid = esort[np.where(mask, idx, 0)]
                    cm = np.where(mask, cols_b[eid], 0)          # [128, dh]
                    vxm = np.where(mask, gxv[b][eid], 0.0)
                    vym = np.where(mask, gyv[b][eid], 0.0)
                    pad = ((0, G - 128), (0, 0))
                    cmp_ = np.pad(cm, pad).reshape(T, R * dh)
                    col_stream[toff:toff + T, hh] = np.pad(
                        cmp_, ((0, 0), (0, 128 - R * dh)))
                    vxm = np.pad(vxm, pad).reshape(T, R, dh)
                    vym = np.pad(vym, pad).reshape(T, R, dh)
                    sel_t[:, hh, e, eis] = vxm[:, eis, ed] * emask
                    sel_t[:, hh, e, R + eis] = vym[:, eis, ed] * emask
                selxy[:, soff:soff + 4 * T * R] = (
                    sel_t.transpose(2, 0, 1, 3).reshape(128, T * 4 * R)
                    .astype(f8np))

            evg = np.ascontiguousarray(
                ev8[col_stream].transpose(2, 0, 1, 3))   # [128, TT, 2, K]

            if USE_COLLECTIVE:
                nown = APAD // 2
                ax_core = np.ascontiguousarray(
                    ax_all[b][:, h * nown:(h + 1) * nown])
            else:
                ax_core = ax_all[b]

            in_maps.append({
                "evg": evg,
                "selxy": selxy,
                "ax": ax_core,
                "evsT": np.ascontiguousarray(
                    (evecs[b][pv].T * valid[None, :]).astype(np.float16)),
                "xinT": np.ascontiguousarray(
                    (x_in[b][pv].T * valid[None, :]).astype(np.float16)),
                "scale": scale,
                "w1t": np.ascontiguousarray(W1.T.reshape(3, P, P).astype(np.float16)),
                "w2t": np.ascontiguousarray(W2.T.astype(np.float16)),
                "w3t": np.ascontiguousarray(W3.T.astype(np.float16)),
                "bret": np.ascontiguousarray(B_re.T.astype(np.float16)),
                "bimt": np.ascontiguousarray(B_im.T.astype(np.float16)),
                "b1": b1.reshape(P, 1).copy(),
                "b2": b2.reshape(P, 1).copy(),
                "b3": b3.reshape(P, 1).copy(),
                "ident": np.eye(P, dtype=np.float16),
            })

    meta = {"geo": geo, "TT": TT, "SELTOT": SELTOT, "d_slots": d_slots}
    return in_maps, core_perm, meta


# ------------------------------------------------------------ device kernel


def build_bass(meta):
    geo = meta["geo"]
    TT = meta["TT"]
    SELTOT = meta["SELTOT"]

    nc = bass.Bass("TRN2", target_bir_lowering=False, debug=False,
                   num_devices=NCORES)

    NAX = APAD // 2 if USE_COLLECTIVE else APAD
    evg_d = nc.dram_tensor("evg", [128, TT, 2, K], f8, kind="ExternalInput")
    selxy_d = nc.dram_tensor("selxy", [128, SELTOT], f8, kind="ExternalInput")
    ax_d = nc.dram_tensor("ax", [128, NAX, 2, P], f16, kind="ExternalInput")
    evsT_d = nc.dram_tensor("evsT", [K, ROWS], f16, kind="ExternalInput")
    xinT_d = nc.dram_tensor("xinT", [P, ROWS], f16, kind="ExternalInput")
    scale_d = nc.dram_tensor("scale", [K, P], f32, kind="ExternalInput")
    w1t_d = nc.dram_tensor("w1t", [3, P, P], f16, kind="ExternalInput")
    w2t_d = nc.dram_tensor("w2t", [P, P], f16, kind="ExternalInput")
    w3t_d = nc.dram_tensor("w3t", [P, P], f16, kind="ExternalInput")
    bret_d = nc.dram_tensor("bret", [P, P], f16, kind="ExternalInput")
    bimt_d = nc.dram_tensor("bimt", [P, P], f16, kind="ExternalInput")
    b1_d = nc.dram_tensor("b1", [P, 1], f32, kind="ExternalInput")
    b2_d = nc.dram_tensor("b2", [P, 1], f32, kind="ExternalInput")
    b3_d = nc.dram_tensor("b3", [P, 1], f32, kind="ExternalInput")
    ident_d = nc.dram_tensor("ident", [P, P], f16, kind="ExternalInput")
    outT_d = nc.dram_tensor("outT", [P, ROWS], f16, kind="ExternalOutput")

    AF = mybir.ActivationFunctionType
    DR = mybir.MatmulPerfMode.DoubleRow
    XCH = 16       # evg DoubleRow tiles per DMA chunk
    EVG_BUFS = 10  # evg ring depth (chunks)
    ACH = 8        # phase-A n-chunks per DMA
    SEL_LA = 5     # selg prefetch lookahead (groups)
    SCR = 64       # agg de-interleave overrun scratch columns
    mxagg = max(T * R for (dh, R, T, _, _) in geo)
    mxsel = max(4 * T * R for (dh, R, T, _, _) in geo)
    n_axdma = (NAX + ACH - 1) // ACH
    A_ITERS = 6
    A_PER_ITER = (NAX + A_ITERS - 1) // A_ITERS
    groups = list(range(0, NBLK, GRP))

    with FixedTileContext(nc) as tc:
        with (
            tc.tile_pool(name="consts", bufs=1) as cpool,
            tc.tile_pool(name="big", bufs=1) as bigpool,
            tc.tile_pool(name="pAx", bufs=8) as pAx,
            tc.tile_pool(name="pX", bufs=EVG_BUFS) as pX,
            tc.tile_pool(name="pS", bufs=SEL_LA + 2) as pS,
            tc.tile_pool(name="pEv", bufs=4) as pEv,
            tc.tile_pool(name="pG", bufs=2) as pG,
            tc.tile_pool(name="psP", bufs=1, space="PSUM") as psP,
            tc.tile_pool(name="dram", bufs=1, space="DRAM") as dpool,
        ):
            # ---------------- constants (scalar queue)
            scale_t = cpool.tile([K, P], f32, tag="scale")
            nc.scalar.dma_start(scale_t[:], scale_d[:])
            wh = cpool.tile([P, 7, P], f16, tag="wh")
            nc.scalar.dma_start(wh[:, 0:3, :], w1t_d[:].rearrange("s p q -> p s q"))
            nc.scalar.dma_start(wh[:, 3, :], w2t_d[:])
            nc.scalar.dma_start(wh[:, 4, :], w3t_d[:])
            nc.scalar.dma_start(wh[:, 5, :], bret_d[:])
            nc.scalar.dma_start(wh[:, 6, :], bimt_d[:])
            w1t_t = wh[:, 0:3, :]
            w2t_t = wh[:, 3, :]
            w3t_t = wh[:, 4, :]
            bret_t = wh[:, 5, :]
            bimt_t = wh[:, 6, :]
            b1_t = cpool.tile([P, 1], f32, tag="b1")
            nc.scalar.dma_start(b1_t[:], b1_d[:])
            b2_t = cpool.tile([P, 1], f32, tag="b2")
            nc.scalar.dma_start(b2_t[:], b2_d[:])
            b3_t = cpool.tile([P, 1], f32, tag="b3")
            nc.scalar.dma_start(b3_t[:], b3_d[:])
            ident_t = cpool.tile([P, P], f16, tag="ident")
            nc.scalar.dma_start(ident_t[:], ident_d[:])
            s2h_t = cpool.tile([K, P], f16, tag="s2h")
            s2hT_t = cpool.tile([P, K], f16, tag="s2hT")
            mre_t = cpool.tile([K, P], f16, tag="mre")
            mim_t = cpool.tile([K, P], f16, tag="mim")
            xsf_t = cpool.tile([K, P], f32, tag="xsf")

            # ---------------- persistent SBUF
            xdT_t = bigpool.tile([P, ROWS], f16, tag="xdT")
            xinT_t = bigpool.tile([P, ROWS], f16, tag="xinT")
            agg_t = bigpool.tile([K, 2, ROWS + SCR], f16, tag="agg")

            # ---------------- PSUM budget: 8 banks exactly.
            # agXY ring 2x1 bank; GXY 2 banks (shared: psA pre-loop, psB);
            # BXY 2 banks; psHa 1 bank (h1/h2 + one-shot Mre/Mim matmuls);
            # psHb 1 bank (h3 + one-shot s2h transpose).
            psA = psP.tile([K, P], f32, tag="GXY")

            # ---------------- phase A DMA triggers (3 queues, upfront)
            ax_tiles = []
            qs = [nc.sync, nc.scalar, nc.gpsimd]
            for ci in range(n_axdma):
                w = min(ACH, NAX - ci * ACH)
                axt = pAx.tile([128, ACH, 2, P], f16, tag="axA")
                qs[ci % 3].dma_start(axt[:, :w], ax_d[:, ci * ACH:ci * ACH + w])
                ax_tiles.append((axt, w))

            # ---------------- eager evg prefetch (sync queue, ring-paced)
            evg_chunks = []
            for t0 in range(0, TT, XCH):
                w = min(XCH, TT - t0)
                xt = pX.tile([128, XCH, 2, K], f8, tag="evg")
                nc.sync.dma_start(xt[:, :w], evg_d[:, t0:t0 + w])
                evg_chunks.append(xt)

            def evg_tile(t):
                return evg_chunks[t // XCH][:, t % XCH, :, :]

            # ---------------- selg prefetch helper (scalar queue)
            sel_tiles = {}

            def sel_fetch(gi):
                g = groups[gi]
                nb = min(GRP, NBLK - g)
                sel0 = geo[g][4]
                sel1 = (geo[g + nb][4] if g + nb < NBLK else SELTOT)
                selg = pS.tile([128, GRP * mxsel], f8, tag="selg")
                nc.sync.dma_start(selg[:, :sel1 - sel0],
                                   selxy_d[:, sel0:sel1])
                sel_tiles[gi] = selg

            for gi in range(SEL_LA):
                sel_fetch(gi)

            # xinT in 4 chunks on scalar queue (after selg lookahead)
            XQ = (ROWS + 3) // 4
            for ci in range(4):
                w = min(XQ, ROWS - ci * XQ)
                nc.gpsimd.dma_start(xinT_t[:, ci * XQ:ci * XQ + w],
                                    xinT_d[:, ci * XQ:ci * XQ + w])

            # evsT prefetch helper (vector queue)
            evsT_tiles = {}

            def evsT_fetch(gi):
                g0 = groups[gi] * 128
                w = min(512, ROWS - g0)
                evt = pEv.tile([K, 512], f16, tag="evsT")
                nc.gpsimd.dma_start(evt[:, :w], evsT_d[:, g0:g0 + w])
                evsT_tiles[gi] = evt

            evsT_fetch(0)
            evsT_fetch(1)

            # AllReduce bounce buffers
            if USE_COLLECTIVE:
                xs_part = dpool.tile([K, P], f32, tag="xsp")
                xs_red = dpool.tile([K, P], f32, tag="xsr")

            # ---------------- C1 group emission helper
            def emit_c1_slot(gi, q):
                g = groups[gi]
                s = g + q
                dh, R, T, toff, soff = geo[s]
                sel0 = geo[g][4]
                so = soff - sel0
                selg = sel_tiles[gi]
                agXY = psP.tile([K, 2, mxagg], f32, tag="agXY", bufs=2)
                for j in range(T):
                    nc.tensor.matmul(
                        agXY[:, :, j * R:(j + 1) * R],
                        evg_tile(toff + j),
                        selg[:, so + 4 * j * R:so + 4 * (j + 1) * R]
                        .rearrange("k (h x) -> k h x", h=2),
                        start=True, stop=True, perf_mode=DR,
                    )
                dst = agg_t[:, :, s * 128:s * 128 + T * R]
                if q % 2 == 0:
                    nc.vector.tensor_copy(dst, agXY[:, :, :T * R])
                else:
                    nc.scalar.copy(dst, agXY[:, :, :T * R])

            # ---------------- main software-pipelined loop
            pipe_xg = [None] * 5
            pipe_t3 = [None] * 2
            pipe_h1 = [None] * 3
            pipe_h2 = [None] * 3
            a_done = 0
            a_mm = 0

            def emit_a_chunks(budget):
                nonlocal a_done, a_mm
                end = min(NAX, a_done + budget)
                while a_done < end:
                    ci = a_done // ACH
                    axt, w = ax_tiles[ci]
                    i = a_done - ci * ACH
                    take = min(w - i, end - a_done)
                    for j in range(i, i + take):
                        nc.tensor.matmul(
                            psA[:], axt[:, j, 0, :], axt[:, j, 1, :],
                            start=(a_mm == 0), stop=(a_mm == NAX - 1),
                        )
                        a_mm += 1
                    a_done += take

            for it in range(NG + LAGC + 3):
                gi = it                      # C1 group
                hB = it - LAGC               # B / C2D / t12 / tanh group
                hE1 = hB - 1                 # E h1 + relu1
                hE2 = hB - 2                 # E h2 + relu2
                hE3 = hB - 3                 # E h3 + residual + store

                # prefetch triggers
                if gi + SEL_LA < NG:
                    sel_fetch(gi + SEL_LA)
                ge = it - LAGC + 2
                if 2 <= ge < NG:
                    evsT_fetch(ge)

                def gw_of(h):
                    nb = min(GRP, NBLK - groups[h])
                    return nb * 128

                # ---- B (emitted at end of prev iter conceptually; here first
                # is fine: psB shares GXY tag ring, see below)
                # ---- C2D block for group hB
                if 0 <= hB < NG:
                    gw = gw_of(hB)
                    h0 = groups[hB] * 128
                    psGXY = psP.tile([P, 2, GRP * 128], f32, tag="GXY")
                    nc.tensor.matmul(psGXY[:, 0, :gw], s2h_t[:],
                                     agg_t[:, 0, h0:h0 + gw],
                                     start=True, stop=True)
                    nc.tensor.matmul(psGXY[:, 1, :gw], s2h_t[:],
                                     agg_t[:, 1, h0:h0 + gw],
                                     start=True, stop=True)
                    psBXY = psP.tile([P, 2, GRP * 128], f32, tag="BXY")
                    nc.tensor.matmul(psBXY[:, 0, :gw], mre_t[:],
                                     agg_t[:, 0, h0:h0 + gw],
                                     start=True, stop=True)
                    nc.tensor.matmul(psBXY[:, 1, :gw], mim_t[:],
                                     agg_t[:, 1, h0:h0 + gw],
                                     start=True, stop=True)
                    # D elementwise (2^-10 folded so f16 can't overflow):
                    # gxy = 2^-10 gx ; t12 = gxy*bxy ; t3 = tx+ty ;
                    # xg = tanh(1024 * t3)
                    gxy_sb = pG.tile([P, 2, GRP * 128], f16, tag="gxy")
                    nc.vector.tensor_scalar_mul(gxy_sb[:, :, :gw],
                                                psGXY[:, :, :gw], 2.0 ** -10)
                    t12 = pG.tile([P, 2, GRP * 128], f16, tag="t12")
                    nc.vector.tensor_mul(t12[:, :, :gw], gxy_sb[:, :, :gw],
                                         psBXY[:, :, :gw])
                    t3 = pG.tile([P, GRP * 128], f16, tag="t3")
                    nc.gpsimd.tensor_add(t3[:, :gw], t12[:, 0, :gw],
                                         t12[:, 1, :gw])
                    pipe_t3[hB % 2] = t3

                # ---- A-phase matmuls first (critical path to s2)
                if it < A_ITERS:
                    emit_a_chunks(A_PER_ITER)

                # ---- C1 first two slots
                if gi < NG:
                    nb = min(GRP, NBLK - groups[gi])
                    for q in range(min(2, nb)):
                        emit_c1_slot(gi, q)
                if it == A_ITERS:
                    # x_spec done: reduce across the pair, derive s2/Mre/Mim
                    if USE_COLLECTIVE:
                        xs_loc = cpool.tile([K, P], f32, tag="xsl")
                        nc.vector.tensor_copy(xs_loc[:], psA[:])
                        nc.gpsimd.dma_start(xs_part[:], xs_loc[:])
                        nc.gpsimd.collective_compute(
                            "AllReduce",
                            mybir.AluOpType.add,
                            replica_groups=[[0, 1], [2, 3], [4, 5], [6, 7]],
                            ins=[xs_part.opt()],
                            outs=[xs_red.opt()],
                        )
                        nc.gpsimd.dma_start(xsf_t[:], xs_red[:])
                        nc.vector.tensor_mul(s2h_t[:], scale_t[:], xsf_t[:])
                    else:
                        nc.vector.tensor_mul(s2h_t[:], scale_t[:], psA[:])
                    # s2^T via PE transpose, then Mre = s2 Bre^T, Mim = s2 Bim^T
                    trp = psP.tile([P, K], f16, tag="psHb")
                    nc.tensor.transpose(trp[:], s2h_t[:], ident_t[:])
                    nc.vector.tensor_copy(s2hT_t[:], trp[:])
                    mmp = psP.tile([K, P], f32, tag="psHa")
                    nc.tensor.matmul(mmp[:], s2hT_t[:], bret_t[:],
                                     start=True, stop=True)
                    nc.vector.tensor_copy(mre_t[:], mmp[:])
                    mmp2 = psP.tile([K, P], f32, tag="psHa")
                    nc.tensor.matmul(mmp2[:], s2hT_t[:], bimt_t[:],
                                     start=True, stop=True)
                    nc.vector.tensor_copy(mim_t[:], mmp2[:])

                # ---- E h1 for group hE1
                if 0 <= hE1 < NG:
                    gw = gw_of(hE1)
                    g0 = groups[hE1] * 128
                    psH1 = psP.tile([P, GRP * 128], f32, tag="psHa")
                    nc.tensor.matmul(psH1[:, :gw], w1t_t[:, 0, :],
                                     xinT_t[:, g0:g0 + gw],
                                     start=True, stop=False)
                    nc.tensor.matmul(psH1[:, :gw], w1t_t[:, 1, :],
                                     xdT_t[:, g0:g0 + gw],
                                     start=False, stop=False)
                    nc.tensor.matmul(psH1[:, :gw], w1t_t[:, 2, :],
                                     pipe_xg[hE1 % 5][:, :gw],
                                     start=False, stop=True)
                    h_sb = pG.tile([P, GRP * 128], f16, tag="h", bufs=3)
                    nc.scalar.activation(h_sb[:, :gw], psH1[:, :gw], AF.Relu,
                                         bias=b1_t[:])
                    pipe_h1[hE1 % 3] = h_sb

                # ---- C1 third slot
                if gi < NG:
                    nb = min(GRP, NBLK - groups[gi])
                    if nb >= 3:
                        emit_c1_slot(gi, 2)

                # ---- E h2 for group hE1 (same iter, after a C1 gap)
                if 0 <= hE1 < NG:
                    gw = gw_of(hE1)
                    psH2 = psP.tile([P, GRP * 128], f32, tag="psHa")
                    nc.tensor.matmul(psH2[:, :gw], w2t_t[:],
                                     pipe_h1[hE1 % 3][:, :gw],
                                     start=True, stop=True)
                    h2_sb = pG.tile([P, GRP * 128], f16, tag="h2", bufs=3)
                    nc.scalar.activation(h2_sb[:, :gw], psH2[:, :gw], AF.Relu,
                                         bias=b2_t[:])
                    pipe_h2[hE1 % 3] = h2_sb

                # ---- tanh for group hB (deferred so relu1/relu2 go first on
                # the scalar queue; xg(hB) is only needed by h1 next iter)
                if 0 <= hB < NG:
                    gw = gw_of(hB)
                    xg_sb = pG.tile([P, GRP * 128], f16, tag="xg", bufs=5)
                    nc.scalar.activation(xg_sb[:, :gw], pipe_t3[hB % 2][:, :gw],
                                         AF.Tanh, scale=1024.0)
                    pipe_xg[hB % 5] = xg_sb

                # ---- C1 fourth slot
                if gi < NG:
                    nb = min(GRP, NBLK - groups[gi])
                    if nb >= 4:
                        emit_c1_slot(gi, 3)

                # ---- E h3 + residual + store for group hE2 (one more lag)
                if 0 <= hE2 < NG:
                    gw = gw_of(hE2)
                    g0 = groups[hE2] * 128
                    psH3 = psP.tile([P, GRP * 128], f32, tag="psHb")
                    nc.tensor.matmul(psH3[:, :gw], w3t_t[:],
                                     pipe_h2[hE2 % 3][:, :gw],
                                     start=True, stop=True)
                    out_sb = pG.tile([P, GRP * 128], f16, tag="out", bufs=3)
                    nc.vector.scalar_tensor_tensor(
                        out_sb[:, :gw], psH3[:, :gw], b3_t[:],
                        xinT_t[:, g0:g0 + gw],
                        op0=mybir.AluOpType.add, op1=mybir.AluOpType.add)
                    nc.sync.dma_start(outT_d[:, g0:g0 + gw], out_sb[:, :gw])

                # ---- B for group hB+1 (end of iter; psB shares GXY tag)
                hBn = hB + 1
                if 0 <= hBn < NG:
                    gw = gw_of(hBn)
                    psB = psP.tile([P, 2, GRP * 128], f32, tag="GXY")
                    nc.tensor.matmul(psB[:, 0, :gw], s2h_t[:],
                                     evsT_tiles[hBn][:, :gw],
                                     start=True, stop=True)
                    g0 = groups[hBn] * 128
                    nc.scalar.copy(xdT_t[:, g0:g0 + gw], psB[:, 0, :gw])

    return nc


# ---------------------------------------------------------------- top level

_CACHE = {}


def _get_bass(meta):
    key = tuple(meta["d_slots"].tolist())
    if key not in _CACHE:
        _CACHE[key] = build_bass(meta)
    return _CACHE[key]


def kernel(_trace=False, **inputs):
    in_maps, core_perm, meta = build_host_data(inputs)
    nc = _get_bass(meta)
    res = bass_utils.run_bass_kernel_spmd(
        nc, in_maps, core_ids=list(range(NCORES)), trace=_trace,
        trace_cores=list(range(NCORES)) if _trace else None,
    )
    out = np.zeros((B, N, P), np.float32)
    for c in range(NCORES):
        b = c // 2
        perm = core_perm[c]
        valid = perm >= 0
        outT = res.results[c]["outT"]           # [P, ROWS]
        out[b, perm[valid]] = np.asarray(outT, np.float32).T[valid]
    if _trace:
        return out, res
    return out
